# revision 12
# baseline (speedup 1.0000x reference)
"""Trainium2 Bass kernel for nn_MultiHeadAttention (B=2, S=4096, D=512, H=8, DK=DV=64).

Returns (y, attn_flat) like the reference:
  y         [2, 4096, 512]  f32   (LayerNorm(attn_out @ Wo.T + bo + q))
  attn_flat [16, 4096, 4096] f32  (softmax attention probs, head-major)

Sharding: 8 cores; core c handles batch b = c // 4 and query rows
[(c%4)*1024, (c%4+1)*1024).  Attention is fully local per core (each core
holds all heads' K/V for its batch); no collectives.

Pipeline per core:
  phase A: project qhT [hd, sq], khT [hd, sk] (transposed layouts) and
           vh [sk, hv] (natural) from host-pre-transposed q/k/v.
  phase B: per (head, sq-tile of 128): S = qhT.T @ khT (+ key-pad mask as an
           extra accumulated rank-8 matmul), exp on ACT straight out of PSUM
           with per-row accumulation (softmax sums), normalize on DVE, DMA the
           2 MB P tile to HBM; PE-transpose P in 128x128 blocks and run the
           P@V matmul off the transposed chunks; o^T spilled to a scratch DRAM
           buffer.
  phase C: out-projection from o^T, + bias + residual, LayerNorm, write y.
"""

import sys

import numpy as np

try:  # concourse ships in the container image
    import concourse.bass as bass  # noqa: F401
except Exception:  # pragma: no cover
    sys.path.insert(0, "/opt/trn_rl_repo")

import concourse.bass as bass
import concourse.mybir as mybir
import concourse.tile as tile
from concourse import bacc
from concourse.bass_utils import run_bass_kernel_spmd
from concourse.masks import make_identity

B, S, D = 2, 4096, 512
H, DK, DV = 8, 64, 64
LN_EPS = 1e-5
NCORES = 8
CORES_PER_BATCH = NCORES // B  # 4
SQ = S // CORES_PER_BATCH  # 1024 query rows per core
NEG = -1.0e30

F32 = mybir.dt.float32
F32R = mybir.dt.float32r

# toggles (perf/accuracy experiments)
USE_F32R = True  # fast fp32 matmul mode for the PE
TRANS_F32R = False  # float32r PE transposes (1.5 vs 2.0 cyc/row)


MMDT = F32R if USE_F32R else F32  # dtype for tensors feeding PE matmuls


_NC_CACHE = {}


def _build_nc(masked_chunks: tuple, general_mask: bool):
    """Build the single-core Bass program (same NEFF runs SPMD on all 8)."""
    from contextlib import ExitStack

    nc = bacc.Bacc("TRN2", target_bir_lowering=False, debug=False,
                   enable_asserts=False, num_devices=NCORES)

    AF = mybir.ActivationFunctionType
    AX = mybir.AxisListType
    OP = mybir.AluOpType

    # ---- DRAM I/O ----------------------------------------------------------
    q_s = nc.dram_tensor("q_s", [SQ, D], F32, kind="ExternalInput")
    q_t = nc.dram_tensor("q_t", [D, SQ], MMDT, kind="ExternalInput")
    k_t = nc.dram_tensor("k_t", [D, S], MMDT, kind="ExternalInput")
    v_t = nc.dram_tensor("v_t", [D, S], MMDT, kind="ExternalInput")
    wqt = nc.dram_tensor("wqt", [D, D], MMDT, kind="ExternalInput")  # [d, hd]
    wkt = nc.dram_tensor("wkt", [D, D], MMDT, kind="ExternalInput")  # [d, hd]
    wvt = nc.dram_tensor("wvt", [D, D], MMDT, kind="ExternalInput")  # [d, hv]
    wot = nc.dram_tensor("wot", [D, D], MMDT, kind="ExternalInput")  # [hv, d]
    bq_c = nc.dram_tensor("bq_c", [D, 1], F32, kind="ExternalInput")
    bk_c = nc.dram_tensor("bk_c", [D, 1], F32, kind="ExternalInput")
    bv_r = nc.dram_tensor("bv_r", [1, D], F32, kind="ExternalInput")
    bo_r = nc.dram_tensor("bo_r", [1, D], F32, kind="ExternalInput")
    g_r = nc.dram_tensor("g_r", [1, D], F32, kind="ExternalInput")
    lb_r = nc.dram_tensor("lb_r", [1, D], F32, kind="ExternalInput")
    # key-pad mask bias rows: chunk n (512 keys) lives on partition n. [8, 512]
    mrow = nc.dram_tensor("mrow", [8, S // 8], MMDT, kind="ExternalInput")
    sel = nc.dram_tensor("sel", [8, 8, 128], MMDT, kind="ExternalInput")
    if general_mask:
        nmask = nc.dram_tensor("nmask", [SQ, S], mybir.dt.uint8,
                               kind="ExternalInput")

    attn_o = nc.dram_tensor("attn_o", [H, SQ, S], F32, kind="ExternalOutput")
    y_o = nc.dram_tensor("y_o", [SQ, D], F32, kind="ExternalOutput")
    o_hbm = nc.dram_tensor("o_hbm", [4, 128, SQ], MMDT, kind="Internal")

    ktr = k_t.rearrange("(c p) s -> p c s", p=128)  # [128, 4, 4096]
    vtr = v_t.rearrange("(c p) s -> p c s", p=128)
    qtr = q_t.rearrange("(c p) s -> p c s", p=128)

    n_sq_tiles = SQ // 128  # 8
    P_BUFS = 2 if general_mask else 3

    with tile.TileContext(nc) as tc, ExitStack() as ctx:
        # ---- whole-kernel pools -------------------------------------------
        persist = ctx.enter_context(tc.tile_pool(name="persist", bufs=1))
        const = ctx.enter_context(tc.tile_pool(name="const", bufs=1))
        sp = ctx.enter_context(tc.tile_pool(name="sp", bufs=3, space="PSUM"))
        tp = ctx.enter_context(tc.tile_pool(name="tp", bufs=1, space="PSUM"))
        op_ = ctx.enter_context(tc.tile_pool(name="op", bufs=1, space="PSUM"))

        # persistent SBUF tensors
        qhT = [persist.tile([128, SQ], MMDT, name=f"qhT{t}") for t in range(4)]
        khT = [persist.tile([128, S], MMDT, name=f"khT{t}") for t in range(4)]
        vh = persist.tile([128, S // 128, 512], MMDT, name="vh")

        # constants
        ident = const.tile([128, 128], F32)
        make_identity(nc, ident)
        bq_sb = const.tile([128, 4], F32)
        nc.gpsimd.dma_start(out=bq_sb, in_=bq_c.rearrange("(c p) o -> p (c o)", p=128))
        bk_sb = const.tile([128, 4], F32)
        nc.gpsimd.dma_start(out=bk_sb, in_=bk_c.rearrange("(c p) o -> p (c o)", p=128))
        eps_t = const.tile([128, 1], F32)
        nc.vector.memset(eps_t, LN_EPS)
        mrow_sb = const.tile([8, S // 8], MMDT)
        nc.gpsimd.dma_start(out=mrow_sb, in_=mrow[:, :])
        sel_sb = const.tile([8, 8, 128], MMDT)
        nc.gpsimd.dma_start(out=sel_sb, in_=sel[:, :, :])

        t_ps = tp.tile([128, 512], F32, name="t_ps")  # transpose staging

        def pe_touch(col_ap):
            """Dead 1x1 PE transpose reading col_ap [P,1]: advances the PE
            vector clock past col_ap's producer so the next real matmul
            carries at most one semaphore wait (fp32 LW struct limit)."""
            if col_ap.dtype != F32:
                col_ap = col_ap.bitcast(F32)
            p = col_ap.partition_size()
            nc.tensor.transpose(t_ps[0:1, 0:1], col_ap, ident[0:p, 0:1])

        pe_touch(ident[:, 0:1])
        pe_touch(sel_sb[:, 0, 0:1])
        pe_touch(mrow_sb[:, 0:1])

        # ---- phase A: projections -----------------------------------------
        with tc.tile_pool(name="io", bufs=4) as io, \
             tc.tile_pool(name="wp", bufs=2) as wp:
            bv_b = wp.tile([128, D], F32, tag="bvb", name="bv_b", bufs=1)
            nc.gpsimd.dma_start(out=bv_b, in_=bv_r[0:1, :].partition_broadcast(128))
            # qhT and khT:  out[hd_tile, s_chunk] = wxt[:, hd].T @ x_t[:, s]
            for (w_dram, b_sb, outs, src_r, nfree) in (
                (wqt, bq_sb, qhT, qtr, SQ),
                (wkt, bk_sb, khT, ktr, S),
            ):
                w_sb = wp.tile([128, 4, 512], MMDT, tag="w", name="w_sb")
                nc.sync.dma_start(out=w_sb, in_=w_dram.rearrange("(c p) h -> p c h", p=128))
                pe_touch(w_sb[:, 0, 0:1])
                for half in range(max(1, nfree // 2048)):
                    hw = min(2048, nfree)
                    ios = []
                    for c in range(4):
                        io_c = io.tile([128, 2048], MMDT, tag="io", name="io_c")
                        nc.sync.dma_start(out=io_c[:, :hw],
                                          in_=src_r[:, c, half * 2048:half * 2048 + hw])
                        ios.append(io_c)
                    for t in range(4):
                        for n in range(hw // 512):
                            ps = sp.tile([128, 512], F32, tag="sp", name="ps_proj")
                            for c in range(4):
                                nc.tensor.matmul(
                                    ps, (w_sb[:, c, t * 128:(t + 1) * 128]),
                                    (ios[c][:, n * 512:(n + 1) * 512]),
                                    start=(c == 0), stop=(c == 3))
                            nc.scalar.activation(
                                out=outs[t][:, half * 2048 + n * 512:half * 2048 + (n + 1) * 512],
                                in_=ps, func=AF.Identity,
                                bias=b_sb[:, t:t + 1], scale=1.0)
            # vh: out[sk_tile, hv] = v_t[:, sk_block].T @ wvt
            w_sb = wp.tile([128, 4, 512], MMDT, tag="w", name="w_sb_v")
            nc.sync.dma_start(out=w_sb, in_=wvt.rearrange("(c p) h -> p c h", p=128))
            pe_touch(w_sb[:, 0, 0:1])
            for half in range(2):
                ios = []
                for c in range(4):
                    io_c = io.tile([128, 2048], MMDT, tag="io", name="io_cv")
                    nc.sync.dma_start(out=io_c, in_=vtr[:, c, half * 2048:(half + 1) * 2048])
                    ios.append(io_c)
                for s16 in range(16):
                    s = half * 16 + s16
                    ps = sp.tile([128, 512], F32, tag="sp", name="ps_v")
                    for c in range(4):
                        nc.tensor.matmul(
                            ps, (ios[c][:, s16 * 128:(s16 + 1) * 128]),
                            (w_sb[:, c, :]),
                            start=(c == 0), stop=(c == 3))
                    nc.vector.tensor_add(vh[:, s, :], ps, bv_b)

        # ---- phase B: attention -------------------------------------------
        with tc.tile_pool(name="pb", bufs=P_BUFS) as pb, \
             tc.tile_pool(name="ptc", bufs=4) as ptc, \
             tc.tile_pool(name="sm", bufs=4) as sm, \
             tc.tile_pool(name="ob", bufs=2) as ob, \
             tc.tile_pool(name="nm", bufs=1) as nmp:

            for h in range(H):
                ht, hr = h // 2, (h % 2) * 64
                p_tiles = {}
                for j in range(n_sq_tiles // 2):
                    for i in (2 * j, 2 * j + 1):
                        # --- scores + softmax for sq-tile i ----------------
                        p_sb = pb.tile([128, S], F32, tag="p", name="p_sb")
                        sums = sm.tile([128, 4], F32, tag="sums", name="sums")
                        for m in range(4):  # 1024-wide psum chunks
                            ps = sp.tile([128, 1024], F32, tag="sp", name="ps_qk")
                            for half in range(2):
                                n = 2 * m + half
                                sl = slice(half * 512, (half + 1) * 512)
                                masked = n in masked_chunks
                                nc.tensor.matmul(
                                    ps[:, sl],
                                    (qhT[ht][hr:hr + 64, i * 128:(i + 1) * 128]),
                                    (khT[ht][hr:hr + 64, n * 512:(n + 1) * 512]),
                                    start=True, stop=not masked)
                                if masked:
                                    nc.tensor.matmul(
                                        ps[:, sl], (sel_sb[:, n, :]),
                                        (mrow_sb[:, :]),
                                        start=False, stop=True)
                            nc.scalar.activation(
                                out=p_sb[:, m * 1024:(m + 1) * 1024], in_=ps,
                                func=AF.Exp, bias=0.0, scale=float(1.0 / np.sqrt(DK)),
                                accum_out=sums[:, m:m + 1])
                        ssum = sm.tile([128, 1], F32, tag="ssum", name="ssum")
                        if general_mask:
                            nm_f = nmp.tile([128, S], F32, tag="nm", name="nm_f")
                            nm_u8 = nmp.tile([128, S], mybir.dt.uint8, tag="nmu", name="nm_u8")
                            nc.sync.dma_start(out=nm_u8, in_=nmask[i * 128:(i + 1) * 128, :])
                            nc.gpsimd.tensor_copy(out=nm_f, in_=nm_u8)
                            nc.vector.scalar_tensor_tensor(
                                out=p_sb, in0=p_sb, scalar=1.0, in1=nm_f,
                                op0=OP.mult, op1=OP.mult, accum_out=ssum)
                        else:
                            nc.vector.reduce_sum(ssum, sums, axis=AX.X)
                        recip = sm.tile([128, 1], F32, tag="recip", name="recip")
                        nc.vector.reciprocal(recip, ssum)
                        nc.vector.tensor_scalar_mul(p_sb, in0=p_sb, scalar1=recip)
                        nc.sync.dma_start(out=attn_o[h, i * 128:(i + 1) * 128, :], in_=p_sb)
                        p_tiles[i] = p_sb
                    # --- P^T + P@V for sq block j (256 rows) ---------------
                    o_ps = op_.tile([64, 256], F32, name="o_ps")
                    for s in range(S // 128):
                        th = (s % 2) * 256
                        nc.tensor.transpose(
                            t_ps[:, th:th + 128],
                            p_tiles[2 * j][:, s * 128:(s + 1) * 128], ident)
                        nc.tensor.transpose(
                            t_ps[:, th + 128:th + 256],
                            p_tiles[2 * j + 1][:, s * 128:(s + 1) * 128], ident)
                        ptile = ptc.tile([128, 256], MMDT, tag="pt", name="ptile")
                        nc.vector.tensor_copy(out=ptile, in_=t_ps[:, th:th + 256])
                        nc.tensor.matmul(
                            o_ps, (vh[:, s, h * 64:(h + 1) * 64]), (ptile),
                            start=(s == 0), stop=(s == S // 128 - 1))
                    o_sm = ob.tile([64, 256], MMDT, tag="osm", name="o_sm")
                    nc.vector.tensor_copy(out=o_sm, in_=o_ps)
                    nc.sync.dma_start(
                        out=o_hbm[ht, hr:hr + 64, j * 256:(j + 1) * 256], in_=o_sm)

        # ---- phase C: out-projection + residual + LayerNorm ---------------
        with tc.tile_pool(name="cp", bufs=2) as cp, \
             tc.tile_pool(name="cw", bufs=1) as cw:
            wo_sb = cw.tile([128, 4, 512], MMDT, name="wo_sb")
            nc.sync.dma_start(out=wo_sb, in_=wot.rearrange("(c p) d -> p c d", p=128))
            pe_touch(wo_sb[:, 0, 0:1])
            bo_b = cw.tile([128, D], F32, name="bo_b")
            nc.gpsimd.dma_start(out=bo_b, in_=bo_r[0:1, :].partition_broadcast(128))
            g_b = cw.tile([128, D], F32, name="g_b")
            nc.gpsimd.dma_start(out=g_b, in_=g_r[0:1, :].partition_broadcast(128))
            lb_b = cw.tile([128, D], F32, name="lb_b")
            nc.gpsimd.dma_start(out=lb_b, in_=lb_r[0:1, :].partition_broadcast(128))
            ohr = o_hbm.rearrange("c p s -> p c s")
            for i in range(n_sq_tiles):
                oc = cp.tile([128, 4, 128], MMDT, tag="oc", name="oc")
                nc.sync.dma_start(out=oc, in_=ohr[:, :, i * 128:(i + 1) * 128])
                ps = sp.tile([128, 512], F32, tag="sp", name="ps_o")
                for c in range(4):
                    nc.tensor.matmul(ps, (oc[:, c, :]), (wo_sb[:, c, :]),
                                     start=(c == 0), stop=(c == 3))
                qres = cp.tile([128, D], F32, tag="qres", name="qres")
                nc.sync.dma_start(out=qres, in_=q_s[i * 128:(i + 1) * 128, :])
                x = cp.tile([128, D], F32, tag="x", name="x")
                nc.vector.tensor_add(x, ps, qres)
                nc.vector.tensor_add(x, x, bo_b)
                stats = cp.tile([128, 6], F32, tag="stats", name="stats")
                nc.vector.bn_stats(out=stats, in_=x)
                mv = cp.tile([128, 2], F32, tag="mv", name="mv")
                nc.vector.bn_aggr(out=mv, in_=stats)
                sd = cp.tile([128, 1], F32, tag="sd", name="sd")
                nc.scalar.activation(out=sd, in_=mv[:, 1:2], func=AF.Sqrt,
                                     bias=eps_t, scale=1.0)
                rstd = cp.tile([128, 1], F32, tag="rstd", name="rstd")
                nc.vector.reciprocal(rstd, sd)
                nc.vector.tensor_scalar(out=x, in0=x, scalar1=mv[:, 0:1],
                                        scalar2=rstd, op0=OP.subtract, op1=OP.mult)
                y_t = cp.tile([128, D], F32, tag="y", name="y_t")
                nc.vector.tensor_mul(y_t, x, g_b)
                nc.vector.tensor_add(y_t, y_t, lb_b)
                nc.sync.dma_start(out=y_o[i * 128:(i + 1) * 128, :], in_=y_t)

    nc.compile()
    return nc


def _get_nc(masked_chunks, general_mask):
    key = (tuple(masked_chunks), bool(general_mask), USE_F32R, TRANS_F32R)
    if key not in _NC_CACHE:
        _NC_CACHE[key] = _build_nc(tuple(masked_chunks), bool(general_mask))
    return _NC_CACHE[key]


def kernel(q, k, v, mask, Wq, bq, Wk, bk, Wv, bv, Wo, bo, ln_g, ln_b,
           _want_results=False, _trace=False):
    q = np.asarray(q, np.float32)
    k = np.asarray(k, np.float32)
    v = np.asarray(v, np.float32)
    mask = np.asarray(mask)
    f32 = np.float32

    # host-side prep (cheap vs device work): transposes + mask analysis
    uniform = bool((mask == mask[:, :1, :]).all())
    masked_chunks = ()
    mrow_np = np.zeros((8, S // 8), f32)
    if uniform:
        anym = mask[:, 0, :].any(axis=0)  # union over batches
        masked_chunks = tuple(
            n for n in range(8) if anym[n * 512:(n + 1) * 512].any())
    sel_np = np.zeros((8, 8, 128), f32)
    for n in range(8):
        sel_np[n, n, :] = 1.0

    wqt = np.ascontiguousarray(np.asarray(Wq, f32).T)
    wkt = np.ascontiguousarray(np.asarray(Wk, f32).T)
    wvt = np.ascontiguousarray(np.asarray(Wv, f32).T)
    wot = np.ascontiguousarray(np.asarray(Wo, f32).T)
    common = {
        "wqt": wqt, "wkt": wkt, "wvt": wvt, "wot": wot,
        "bq_c": np.asarray(bq, f32).reshape(D, 1),
        "bk_c": np.asarray(bk, f32).reshape(D, 1),
        "bv_r": np.asarray(bv, f32).reshape(1, D),
        "bo_r": np.asarray(bo, f32).reshape(1, D),
        "g_r": np.asarray(ln_g, f32).reshape(1, D),
        "lb_r": np.asarray(ln_b, f32).reshape(1, D),
        "sel": sel_np,
    }

    in_maps = []
    for c in range(NCORES):
        b = c // CORES_PER_BATCH
        s0 = (c % CORES_PER_BATCH) * SQ
        m = dict(common)
        m["q_s"] = np.ascontiguousarray(q[b, s0:s0 + SQ])
        m["q_t"] = np.ascontiguousarray(q[b, s0:s0 + SQ].T)
        m["k_t"] = np.ascontiguousarray(k[b].T)
        m["v_t"] = np.ascontiguousarray(v[b].T)
        if uniform:
            mr = np.where(mask[b, 0, :], np.float32(NEG), np.float32(0.0))
            m["mrow"] = np.ascontiguousarray(mr.reshape(8, S // 8))
        else:
            m["mrow"] = mrow_np
            m["nmask"] = np.ascontiguousarray(
                (~mask[b, s0:s0 + SQ]).astype(np.uint8))
        in_maps.append(m)

    nc = _get_nc(masked_chunks, not uniform)
    res = run_bass_kernel_spmd(nc, in_maps, core_ids=list(range(NCORES)),
                               trace=_trace)

    y = np.empty((B, S, D), f32)
    attn = np.empty((H * B, S, S), f32)
    for c, r in enumerate(res.results):
        b = c // CORES_PER_BATCH
        s0 = (c % CORES_PER_BATCH) * SQ
        y[b, s0:s0 + SQ] = r["y_o"]
        for h in range(H):
            attn[h * B + b, s0:s0 + SQ, :] = r["attn_o"][h]
    if _want_results:
        return (y, attn), res
    return (y, attn)


# revision 15
# speedup vs baseline: 1.1143x; 1.1143x over previous
"""Trainium2 Bass kernel for nn_MultiHeadAttention (B=2, S=4096, D=512, H=8, DK=DV=64).

Returns (y, attn_flat) like the reference:
  y         [2, 4096, 512]  f32   (LayerNorm(attn_out @ Wo.T + bo + q))
  attn_flat [16, 4096, 4096] f32  (softmax attention probs, head-major)

Sharding: 8 cores; core c handles batch b = c // 4 and query rows
[(c%4)*1024, (c%4+1)*1024).  Attention is fully local per core (each core
holds all heads' K/V for its batch); no collectives.

Pipeline per core:
  phase A: project qhT [hd, sq], khT [hd, sk] (transposed layouts) and
           vh [sk, hv] (natural) from host-pre-transposed q/k/v.
  phase B: per (head, sq-tile of 128): S = qhT.T @ khT (+ key-pad mask as an
           extra accumulated rank-8 matmul), exp on ACT straight out of PSUM
           with per-row accumulation (softmax sums), normalize on DVE, DMA the
           2 MB P tile to HBM; PE-transpose P in 128x128 blocks and run the
           P@V matmul off the transposed chunks; o^T spilled to a scratch DRAM
           buffer.
  phase C: out-projection from o^T, + bias + residual, LayerNorm, write y.
"""

import sys

import numpy as np

try:  # concourse ships in the container image
    import concourse.bass as bass  # noqa: F401
except Exception:  # pragma: no cover
    sys.path.insert(0, "/opt/trn_rl_repo")

import concourse.bass as bass
import concourse.mybir as mybir
import concourse.tile as tile
from concourse import bacc
from concourse.bass_utils import run_bass_kernel_spmd

B, S, D = 2, 4096, 512
H, DK, DV = 8, 64, 64
LN_EPS = 1e-5
NCORES = 8
CORES_PER_BATCH = NCORES // B  # 4
SQ = S // CORES_PER_BATCH  # 1024 query rows per core
NEG = -1.0e30

F32 = mybir.dt.float32
F32R = mybir.dt.float32r

# toggles (perf/accuracy experiments)
USE_F32R = True  # fast fp32 matmul mode for the PE
TRANS_F32R = False  # float32r PE transposes (1.5 vs 2.0 cyc/row)


MMDT = F32R if USE_F32R else F32  # dtype for tensors feeding PE matmuls


_NC_CACHE = {}


def _build_nc(masked_chunks: tuple, general_mask: bool):
    """Build the single-core Bass program (same NEFF runs SPMD on all 8)."""
    from contextlib import ExitStack

    nc = bacc.Bacc("TRN2", target_bir_lowering=False, debug=False,
                   enable_asserts=False, num_devices=NCORES)

    AF = mybir.ActivationFunctionType
    AX = mybir.AxisListType
    OP = mybir.AluOpType

    # ---- DRAM I/O ----------------------------------------------------------
    q_s = nc.dram_tensor("q_s", [SQ, D], F32, kind="ExternalInput")
    q_t = nc.dram_tensor("q_t", [D, SQ], MMDT, kind="ExternalInput")
    k_t = nc.dram_tensor("k_t", [D, S], MMDT, kind="ExternalInput")
    v_t = nc.dram_tensor("v_t", [D, S], MMDT, kind="ExternalInput")
    wqt = nc.dram_tensor("wqt", [D, D], MMDT, kind="ExternalInput")  # [d, hd]
    wkt = nc.dram_tensor("wkt", [D, D], MMDT, kind="ExternalInput")  # [d, hd]
    wvt = nc.dram_tensor("wvt", [D, D], MMDT, kind="ExternalInput")  # [d, hv]
    wot = nc.dram_tensor("wot", [D, D], MMDT, kind="ExternalInput")  # [hv, d]
    bq_c = nc.dram_tensor("bq_c", [D, 1], F32, kind="ExternalInput")
    bk_c = nc.dram_tensor("bk_c", [D, 1], F32, kind="ExternalInput")
    bv_r = nc.dram_tensor("bv_r", [1, D], F32, kind="ExternalInput")
    bo_r = nc.dram_tensor("bo_r", [1, D], F32, kind="ExternalInput")
    g_r = nc.dram_tensor("g_r", [1, D], F32, kind="ExternalInput")
    lb_r = nc.dram_tensor("lb_r", [1, D], F32, kind="ExternalInput")
    # key-pad mask bias rows: chunk n (512 keys) lives on partition n. [8, 512]
    mrow = nc.dram_tensor("mrow", [8, S // 8], MMDT, kind="ExternalInput")
    identf = nc.dram_tensor("identf", [128, 128], F32, kind="ExternalInput")
    sel = nc.dram_tensor("sel", [8, 8, 128], MMDT, kind="ExternalInput")
    if general_mask:
        nmask = nc.dram_tensor("nmask", [SQ, S], mybir.dt.uint8,
                               kind="ExternalInput")

    attn_o = nc.dram_tensor("attn_o", [H, SQ, S], F32, kind="ExternalOutput")
    y_o = nc.dram_tensor("y_o", [SQ, D], F32, kind="ExternalOutput")
    o_hbm = nc.dram_tensor("o_hbm", [4, 128, SQ], MMDT, kind="Internal")

    ktr = k_t.rearrange("(c p) s -> p c s", p=128)  # [128, 4, 4096]
    vtr = v_t.rearrange("(c p) s -> p c s", p=128)
    qtr = q_t.rearrange("(c p) s -> p c s", p=128)

    n_sq_tiles = SQ // 128  # 8
    P_BUFS = 2 if general_mask else 3

    with tile.TileContext(nc) as tc, ExitStack() as ctx:
        # ---- whole-kernel pools -------------------------------------------
        persist = ctx.enter_context(tc.tile_pool(name="persist", bufs=1))
        const = ctx.enter_context(tc.tile_pool(name="const", bufs=1))
        sp = ctx.enter_context(tc.tile_pool(name="sp", bufs=2, space="PSUM"))
        tp = ctx.enter_context(tc.tile_pool(name="tp", bufs=3, space="PSUM"))
        op_ = ctx.enter_context(tc.tile_pool(name="op", bufs=1, space="PSUM"))

        # persistent SBUF tensors
        qhT = [persist.tile([128, SQ], MMDT, name=f"qhT{t}") for t in range(4)]
        khT = [persist.tile([128, S], MMDT, name=f"khT{t}") for t in range(4)]
        vh = persist.tile([128, S // 128, 512], MMDT, name="vh")

        # constants
        ident = const.tile([128, 128], F32)
        nc.gpsimd.dma_start(out=ident, in_=identf[:, :])
        bq_sb = const.tile([128, 4], F32)
        nc.gpsimd.dma_start(out=bq_sb, in_=bq_c.rearrange("(c p) o -> p (c o)", p=128))
        bk_sb = const.tile([128, 4], F32)
        nc.gpsimd.dma_start(out=bk_sb, in_=bk_c.rearrange("(c p) o -> p (c o)", p=128))
        eps_t = const.tile([128, 1], F32)
        nc.vector.memset(eps_t, LN_EPS)
        mrow_sb = const.tile([8, S // 8], MMDT)
        nc.gpsimd.dma_start(out=mrow_sb, in_=mrow[:, :])
        sel_sb = const.tile([8, 8, 128], MMDT)
        nc.gpsimd.dma_start(out=sel_sb, in_=sel[:, :, :])

        def pe_touch(col_ap):
            """Dead 1x1 PE transpose reading col_ap [P,1]: advances the PE
            vector clock past col_ap's producer so the next real matmul
            carries at most one semaphore wait (fp32 LW struct limit)."""
            if col_ap.dtype != F32:
                col_ap = col_ap.bitcast(F32)
            p = col_ap.partition_size()
            tt = tp.tile([128, 512], F32, tag="t", name="touch")
            nc.tensor.transpose(tt[0:1, 0:1], col_ap, ident[0:p, 0:1])

        pe_touch(ident[:, 0:1])
        pe_touch(sel_sb[:, 0, 0:1])
        pe_touch(mrow_sb[:, 0:1])

        # ---- phase A: projections -----------------------------------------
        with tc.tile_pool(name="io", bufs=4) as io, \
             tc.tile_pool(name="wp", bufs=2) as wp:
            bv_b = wp.tile([128, D], F32, tag="bvb", name="bv_b", bufs=1)
            nc.gpsimd.dma_start(out=bv_b, in_=bv_r[0:1, :].partition_broadcast(128))
            # qhT and khT:  out[hd_tile, s_chunk] = wxt[:, hd].T @ x_t[:, s]
            for (w_dram, b_sb, outs, src_r, nfree) in (
                (wqt, bq_sb, qhT, qtr, SQ),
                (wkt, bk_sb, khT, ktr, S),
            ):
                w_sb = wp.tile([128, 4, 512], MMDT, tag="w", name="w_sb")
                nc.sync.dma_start(out=w_sb, in_=w_dram.rearrange("(c p) h -> p c h", p=128))
                pe_touch(w_sb[:, 0, 0:1])
                for half in range(max(1, nfree // 2048)):
                    hw = min(2048, nfree)
                    ios = []
                    for c in range(4):
                        io_c = io.tile([128, 2048], MMDT, tag="io", name="io_c")
                        nc.sync.dma_start(out=io_c[:, :hw],
                                          in_=src_r[:, c, half * 2048:half * 2048 + hw])
                        ios.append(io_c)
                    for t in range(4):
                        for n in range(hw // 512):
                            ps = sp.tile([128, 512], F32, tag="sp", name="ps_proj")
                            for c in range(4):
                                nc.tensor.matmul(
                                    ps, (w_sb[:, c, t * 128:(t + 1) * 128]),
                                    (ios[c][:, n * 512:(n + 1) * 512]),
                                    start=(c == 0), stop=(c == 3))
                            nc.scalar.activation(
                                out=outs[t][:, half * 2048 + n * 512:half * 2048 + (n + 1) * 512],
                                in_=ps, func=AF.Identity,
                                bias=b_sb[:, t:t + 1], scale=1.0)
            # vh: out[sk_tile, hv] = v_t[:, sk_block].T @ wvt
            w_sb = wp.tile([128, 4, 512], MMDT, tag="w", name="w_sb_v")
            nc.sync.dma_start(out=w_sb, in_=wvt.rearrange("(c p) h -> p c h", p=128))
            pe_touch(w_sb[:, 0, 0:1])
            for half in range(2):
                ios = []
                for c in range(4):
                    io_c = io.tile([128, 2048], MMDT, tag="io", name="io_cv")
                    nc.sync.dma_start(out=io_c, in_=vtr[:, c, half * 2048:(half + 1) * 2048])
                    ios.append(io_c)
                for s16 in range(16):
                    s = half * 16 + s16
                    ps = sp.tile([128, 512], F32, tag="sp", name="ps_v")
                    for c in range(4):
                        nc.tensor.matmul(
                            ps, (ios[c][:, s16 * 128:(s16 + 1) * 128]),
                            (w_sb[:, c, :]),
                            start=(c == 0), stop=(c == 3))
                    nc.vector.tensor_add(vh[:, s, :], ps, bv_b)

        # ---- phase B: attention -------------------------------------------
        with tc.tile_pool(name="pb", bufs=P_BUFS) as pb, \
             tc.tile_pool(name="ptc", bufs=3) as ptc, \
             tc.tile_pool(name="sm", bufs=4) as sm, \
             tc.tile_pool(name="ob", bufs=2) as ob, \
             tc.tile_pool(name="nm", bufs=1) as nmp:

            for h in range(H):
                ht, hr = h // 2, (h % 2) * 64
                p_tiles = {}

                def qk_tile(i):
                    # --- scores + softmax for sq-tile i --------------------
                    p_sb = pb.tile([128, S], F32, tag="p", name="p_sb")
                    sums = sm.tile([128, 4], F32, tag="sums", name="sums")
                    for m in range(4):  # 1024-wide psum chunks
                        ps = sp.tile([128, 1024], F32, tag="sp", name="ps_qk")
                        for half in range(2):
                            n = 2 * m + half
                            sl = slice(half * 512, (half + 1) * 512)
                            masked = n in masked_chunks
                            nc.tensor.matmul(
                                ps[:, sl],
                                (qhT[ht][hr:hr + 64, i * 128:(i + 1) * 128]),
                                (khT[ht][hr:hr + 64, n * 512:(n + 1) * 512]),
                                start=True, stop=not masked)
                            if masked:
                                nc.tensor.matmul(
                                    ps[:, sl], (sel_sb[:, n, :]),
                                    (mrow_sb[:, :]),
                                    start=False, stop=True)
                        nc.scalar.activation(
                            out=p_sb[:, m * 1024:(m + 1) * 1024], in_=ps,
                            func=AF.Exp, bias=0.0, scale=float(1.0 / np.sqrt(DK)),
                            accum_out=sums[:, m:m + 1])
                    ssum = sm.tile([128, 1], F32, tag="ssum", name="ssum")
                    if general_mask:
                        nm_f = nmp.tile([128, S], F32, tag="nm", name="nm_f")
                        nm_u8 = nmp.tile([128, S], mybir.dt.uint8, tag="nmu", name="nm_u8")
                        nc.sync.dma_start(out=nm_u8, in_=nmask[i * 128:(i + 1) * 128, :])
                        nc.gpsimd.tensor_copy(out=nm_f, in_=nm_u8)
                        nc.vector.scalar_tensor_tensor(
                            out=p_sb, in0=p_sb, scalar=1.0, in1=nm_f,
                            op0=OP.mult, op1=OP.mult, accum_out=ssum)
                    else:
                        nc.vector.reduce_sum(ssum, sums, axis=AX.X)
                    recip = sm.tile([128, 1], F32, tag="recip", name="recip")
                    nc.vector.reciprocal(recip, ssum)
                    nc.vector.tensor_scalar_mul(p_sb, in0=p_sb, scalar1=recip)
                    nc.sync.dma_start(out=attn_o[h, i * 128:(i + 1) * 128, :],
                                      in_=p_sb)
                    p_tiles[i] = p_sb

                def pv_block(j):
                    # --- P^T + P@V for sq block j (256 rows) ---------------
                    o_ps = op_.tile([64, 256], F32, name="o_ps")
                    for s2 in range(S // 256):  # s-pairs
                        s0, s1 = 2 * s2, 2 * s2 + 1
                        t_ps = tp.tile([128, 512], F32, tag="t", name="t_ps")
                        for col, (pt, s) in enumerate(
                                ((p_tiles[2 * j], s0), (p_tiles[2 * j + 1], s0),
                                 (p_tiles[2 * j], s1), (p_tiles[2 * j + 1], s1))):
                            nc.tensor.transpose(
                                t_ps[:, col * 128:(col + 1) * 128],
                                pt[:, s * 128:(s + 1) * 128], ident)
                        ptile = ptc.tile([128, 512], MMDT, tag="pt", name="ptile")
                        nc.vector.tensor_copy(out=ptile, in_=t_ps)
                        for si, s in enumerate((s0, s1)):
                            nc.tensor.matmul(
                                o_ps, (vh[:, s, h * 64:(h + 1) * 64]),
                                (ptile[:, si * 256:(si + 1) * 256]),
                                start=(s == 0), stop=(s == S // 128 - 1))
                    o_sm = ob.tile([64, 256], MMDT, tag="osm", name="o_sm")
                    nc.vector.tensor_copy(out=o_sm, in_=o_ps)
                    nc.sync.dma_start(
                        out=o_hbm[ht, hr:hr + 64, j * 256:(j + 1) * 256], in_=o_sm)

                for step in (("qk", 0), ("qk", 1), ("qk", 2), ("pv", 0),
                             ("qk", 3), ("qk", 4), ("pv", 1),
                             ("qk", 5), ("qk", 6), ("pv", 2),
                             ("qk", 7), ("pv", 3)):
                    if step[0] == "qk":
                        qk_tile(step[1])
                    else:
                        pv_block(step[1])

        # ---- phase C: out-projection + residual + LayerNorm ---------------
        with tc.tile_pool(name="cp", bufs=2) as cp, \
             tc.tile_pool(name="cw", bufs=1) as cw:
            wo_sb = cw.tile([128, 4, 512], MMDT, name="wo_sb")
            nc.sync.dma_start(out=wo_sb, in_=wot.rearrange("(c p) d -> p c d", p=128))
            pe_touch(wo_sb[:, 0, 0:1])
            bo_b = cw.tile([128, D], F32, name="bo_b")
            nc.gpsimd.dma_start(out=bo_b, in_=bo_r[0:1, :].partition_broadcast(128))
            g_b = cw.tile([128, D], F32, name="g_b")
            nc.gpsimd.dma_start(out=g_b, in_=g_r[0:1, :].partition_broadcast(128))
            lb_b = cw.tile([128, D], F32, name="lb_b")
            nc.gpsimd.dma_start(out=lb_b, in_=lb_r[0:1, :].partition_broadcast(128))
            ohr = o_hbm.rearrange("c p s -> p c s")
            for i in range(n_sq_tiles):
                oc = cp.tile([128, 4, 128], MMDT, tag="oc", name="oc")
                nc.sync.dma_start(out=oc, in_=ohr[:, :, i * 128:(i + 1) * 128])
                ps = sp.tile([128, 512], F32, tag="sp", name="ps_o")
                for c in range(4):
                    nc.tensor.matmul(ps, (oc[:, c, :]), (wo_sb[:, c, :]),
                                     start=(c == 0), stop=(c == 3))
                qres = cp.tile([128, D], F32, tag="qres", name="qres")
                nc.sync.dma_start(out=qres, in_=q_s[i * 128:(i + 1) * 128, :])
                x = cp.tile([128, D], F32, tag="x", name="x")
                nc.vector.tensor_add(x, ps, qres)
                nc.vector.tensor_add(x, x, bo_b)
                stats = cp.tile([128, 6], F32, tag="stats", name="stats")
                nc.vector.bn_stats(out=stats, in_=x)
                mv = cp.tile([128, 2], F32, tag="mv", name="mv")
                nc.vector.bn_aggr(out=mv, in_=stats)
                sd = cp.tile([128, 1], F32, tag="sd", name="sd")
                nc.scalar.activation(out=sd, in_=mv[:, 1:2], func=AF.Sqrt,
                                     bias=eps_t, scale=1.0)
                rstd = cp.tile([128, 1], F32, tag="rstd", name="rstd")
                nc.vector.reciprocal(rstd, sd)
                nc.vector.tensor_scalar(out=x, in0=x, scalar1=mv[:, 0:1],
                                        scalar2=rstd, op0=OP.subtract, op1=OP.mult)
                y_t = cp.tile([128, D], F32, tag="y", name="y_t")
                nc.vector.tensor_mul(y_t, x, g_b)
                nc.vector.tensor_add(y_t, y_t, lb_b)
                nc.sync.dma_start(out=y_o[i * 128:(i + 1) * 128, :], in_=y_t)

    nc.compile()
    return nc


def _get_nc(masked_chunks, general_mask):
    key = (tuple(masked_chunks), bool(general_mask), USE_F32R, TRANS_F32R)
    if key not in _NC_CACHE:
        _NC_CACHE[key] = _build_nc(tuple(masked_chunks), bool(general_mask))
    return _NC_CACHE[key]


def kernel(q, k, v, mask, Wq, bq, Wk, bk, Wv, bv, Wo, bo, ln_g, ln_b,
           _want_results=False, _trace=False):
    q = np.asarray(q, np.float32)
    k = np.asarray(k, np.float32)
    v = np.asarray(v, np.float32)
    mask = np.asarray(mask)
    f32 = np.float32

    # host-side prep (cheap vs device work): transposes + mask analysis
    uniform = bool((mask == mask[:, :1, :]).all())
    masked_chunks = ()
    mrow_np = np.zeros((8, S // 8), f32)
    if uniform:
        anym = mask[:, 0, :].any(axis=0)  # union over batches
        masked_chunks = tuple(
            n for n in range(8) if anym[n * 512:(n + 1) * 512].any())
    sel_np = np.zeros((8, 8, 128), f32)
    for n in range(8):
        sel_np[n, n, :] = 1.0

    wqt = np.ascontiguousarray(np.asarray(Wq, f32).T)
    wkt = np.ascontiguousarray(np.asarray(Wk, f32).T)
    wvt = np.ascontiguousarray(np.asarray(Wv, f32).T)
    wot = np.ascontiguousarray(np.asarray(Wo, f32).T)
    common = {
        "wqt": wqt, "wkt": wkt, "wvt": wvt, "wot": wot,
        "bq_c": np.asarray(bq, f32).reshape(D, 1),
        "bk_c": np.asarray(bk, f32).reshape(D, 1),
        "bv_r": np.asarray(bv, f32).reshape(1, D),
        "bo_r": np.asarray(bo, f32).reshape(1, D),
        "g_r": np.asarray(ln_g, f32).reshape(1, D),
        "lb_r": np.asarray(ln_b, f32).reshape(1, D),
        "sel": sel_np,
        "identf": np.eye(128, dtype=f32),
    }

    in_maps = []
    for c in range(NCORES):
        b = c // CORES_PER_BATCH
        s0 = (c % CORES_PER_BATCH) * SQ
        m = dict(common)
        m["q_s"] = np.ascontiguousarray(q[b, s0:s0 + SQ])
        m["q_t"] = np.ascontiguousarray(q[b, s0:s0 + SQ].T)
        m["k_t"] = np.ascontiguousarray(k[b].T)
        m["v_t"] = np.ascontiguousarray(v[b].T)
        if uniform:
            mr = np.where(mask[b, 0, :], np.float32(NEG), np.float32(0.0))
            m["mrow"] = np.ascontiguousarray(mr.reshape(8, S // 8))
        else:
            m["mrow"] = mrow_np
            m["nmask"] = np.ascontiguousarray(
                (~mask[b, s0:s0 + SQ]).astype(np.uint8))
        in_maps.append(m)

    nc = _get_nc(masked_chunks, not uniform)
    res = run_bass_kernel_spmd(nc, in_maps, core_ids=list(range(NCORES)),
                               trace=_trace)

    y = np.empty((B, S, D), f32)
    attn = np.empty((H * B, S, S), f32)
    for c, r in enumerate(res.results):
        b = c // CORES_PER_BATCH
        s0 = (c % CORES_PER_BATCH) * SQ
        y[b, s0:s0 + SQ] = r["y_o"]
        for h in range(H):
            attn[h * B + b, s0:s0 + SQ, :] = r["attn_o"][h]
    if _want_results:
        return (y, attn), res
    return (y, attn)
